# revision 1
# baseline (speedup 1.0000x reference)
"""DialogueGCN forward as a Bass/Tile kernel on 8 TRN2 NeuronCores.

Sharding: data-parallel over dialogues (batch). Each core owns 32 contiguous
dialogues; edges never cross dialogues so all graph aggregation is local.

Per-dialogue math (u = source utterance, t = target utterance, band |u-t|<=10):
  scaleT[u, s] = (W_att^T @ x_b^T)[u, s]            (s = softmax axis)
  P = exp(scaleT)                                    (softmax Z cancels in the
                                                      masked renormalization)
  Shat_{sig,dir}[u,t] = P[u,t] * msk_sig(u) * dir_mask[u,t]   (+ row sums)
  sums[u] = sum_{sig,dir,t} Shat                     (win-masked row sum)
  xr[u,:] = x[u,:] / sums[u]
  G_{sig,dir} = Shat^T-contract: psum[dd,t] += xr[u,dd] * Shat[u,t]
  h1_st[tau] = sum_{sig,dir} w8[sig*4+tau*2+dir]^T @ G_{sig,dir}
  h1 = select_by_target_speaker(h1_st0, h1_st1) + root^T @ x^T + bias_r
  qT[u,h2] = (h1^T W2)[u,h2];  h2 = W1^T h1 + qT^T-contract win + b_gc
  hid = relu(Wlin_d^T x^T + Wlin_h^T h2 + b_lin)
  logits = Wfc^T hid + b_fc;  out = log_softmax over 6 classes
"""

import os

import numpy as np

import concourse.bass as bass
import concourse.mybir as mybir
import concourse.tile as tile
from concourse import bass_utils
SEQ, BATCH, D, H, NCLS = 300, 256, 200, 128, 6
WP = WF = 10
NCORES = 8
BPC = BATCH // NCORES  # dialogues per core
UT = [(0, 128), (128, 128), (256, 44)]  # seq tiles (offset, size)
F32 = mybir.dt.float32
F32R = mybir.dt.float32r

_CACHE = {}


def _split_multiwaits(nc, max_waits=1):
    """walrus in this container rejects >1 sem wait on an instruction
    ("Too many sync wait commands"); hoist extras onto preceding NOPs."""
    n = 0
    for f in nc.m.functions:
        for b in f.blocks:
            newlist = []
            changed = False
            for ins in b.instructions:
                si = ins.sync_info
                if si is not None and si.on_wait is not None and len(si.on_wait) > max_waits:
                    waits = list(si.on_wait)
                    for w in waits[max_waits:]:
                        n += 1
                        nop = mybir.InstNoOp(name=f"waitsplit-{n}", ins=[], outs=[])
                        nop.engine = ins.engine
                        nop.sync_info = mybir.SyncInfo(on_wait=[w], on_update=[])
                        newlist.append(nop)
                        nc.inst_map[nop.name] = nop
                    ins.sync_info = mybir.SyncInfo(
                        on_wait=waits[:max_waits],
                        on_update=list(si.on_update) if si.on_update else [],
                    )
                    changed = True
                newlist.append(ins)
            if changed:
                b.instructions = newlist
    return n


MMDT = F32R  # dtype for matmul-facing tensors (F32R = fast reduced fp32)


def _mmr(nc, out, lhsT, rhs, start, stop):
    nc.tensor.matmul(out, lhsT, rhs, start=start, stop=stop)


def _build_program():
    nc = bass.Bass("TRN2", num_devices=NCORES)

    ap = {}
    def din(name, shape, mm=False):
        dt = MMDT if mm else F32
        ap[name] = nc.dram_tensor(name, shape, dt, kind="ExternalInput").ap()

    din("xt", (BPC, D, SEQ), True)    # per-dialogue x^T (d-major)
    din("xn", (BPC, SEQ, D))          # per-dialogue x (seq-major)
    din("msk", (2, BPC, SEQ))         # speaker one-hot masks
    din("dir0", (SEQ, SEQ))           # band & u<t
    din("dir1", (SEQ, SEQ))           # band & u>=t
    din("winm", (SEQ, SEQ), True)           # band (dir0+dir1)
    din("watt", (2, 100, SEQ), True)        # W_att d-chunks
    din("w8", (8, 2, 100, H), True)         # relation weights, d-chunks
    din("rootm", (2, 100, H), True)
    din("w1m", (H, H), True)
    din("w2m", (H, H), True)
    din("wlind", (2, 100, H), True)
    din("wlinh", (H, H), True)
    din("wfc", (H, NCLS), True)
    din("ident", (128, 128))
    din("brc", (H, 1))
    din("bgc", (H, 1))
    din("blc", (H, 1))
    din("bfc", (NCLS, 1))
    out = nc.dram_tensor("out", (BPC * SEQ, NCLS), F32, kind="ExternalOutput").ap()

    repeat = int(os.environ.get("BASS_REPEAT", "1"))
    from contextlib import ExitStack
    with tile.TileContext(nc) as tc:
        with ExitStack() as ctx:
            pools = _mk_pools(tc, ctx)
            if repeat > 1:
                with tc.For_i(0, repeat, 1):
                    _body(nc, tc, ap, out, pools)
            else:
                _body(nc, tc, ap, out, pools)

    _split_multiwaits(nc)
    return nc


def _mk_pools(tc, ctx):
    return dict(
        cpool=ctx.enter_context(tc.tile_pool(name="const", bufs=1)),
        io=ctx.enter_context(tc.tile_pool(name="io", bufs=2)),
        wk=ctx.enter_context(tc.tile_pool(name="wk", bufs=2)),
        spool=ctx.enter_context(tc.tile_pool(name="spool", bufs=12)),
        gpool=ctx.enter_context(tc.tile_pool(name="gpool", bufs=8)),
        ps_big=ctx.enter_context(tc.tile_pool(name="ps_big", bufs=4, space="PSUM")),
        ps_qt=ctx.enter_context(tc.tile_pool(name="ps_qt", bufs=2, space="PSUM")),
        ps_lt=ctx.enter_context(tc.tile_pool(name="ps_lt", bufs=1, space="PSUM")),
    )


def _body(nc, tc, ap, out, pools):
    cpool = pools["cpool"]
    io = pools["io"]
    wk = pools["wk"]
    spool = pools["spool"]
    gpool = pools["gpool"]
    ps_big = pools["ps_big"]
    ps_qt = pools["ps_qt"]
    ps_lt = pools["ps_lt"]

    # ---- resident constants ----
    sb_dir = {}
    for k, (u0, uk) in enumerate(UT):
        for nm in ("dir0", "dir1", "winm"):
            t = cpool.tile([128, SEQ], MMDT if nm == "winm" else F32, name=f"c_{nm}_{k}")
            nc.sync.dma_start(t[:uk, :], ap[nm][u0:u0 + uk, :])
            sb_dir[(nm, k)] = t
    sb_watt = []
    for ch in range(2):
        t = cpool.tile([100, SEQ], MMDT, name=f"c_watt_{ch}")
        nc.sync.dma_start(t[:], ap["watt"][ch])
        sb_watt.append(t)
    sb_w8 = []
    for ch in range(2):
        t = cpool.tile([100, 8 * H], MMDT, name=f"c_w8_{ch}")
        nc.sync.dma_start(
            t.rearrange("p (r h) -> p r h", r=8),
            ap["w8"][:, ch, :, :].transpose([1, 0, 2]),
        )
        sb_w8.append(t)
    sb_root = cpool.tile([100, 2 * H], MMDT, name="c_root")
    nc.sync.dma_start(
        sb_root.rearrange("p (c h) -> p c h", c=2),
        ap["rootm"].transpose([1, 0, 2]),
    )
    sb_wlind = cpool.tile([100, 2 * H], MMDT, name="c_wlind")
    nc.sync.dma_start(
        sb_wlind.rearrange("p (c h) -> p c h", c=2),
        ap["wlind"].transpose([1, 0, 2]),
    )
    sb_w1 = cpool.tile([H, H], MMDT, name="c_w1")
    nc.sync.dma_start(sb_w1[:], ap["w1m"][:])
    sb_w2 = cpool.tile([H, H], MMDT, name="c_w2")
    nc.sync.dma_start(sb_w2[:], ap["w2m"][:])
    sb_wlinh = cpool.tile([H, H], MMDT, name="c_wlinh")
    nc.sync.dma_start(sb_wlinh[:], ap["wlinh"][:])
    sb_wfc = cpool.tile([H, NCLS], MMDT, name="c_wfc")
    nc.sync.dma_start(sb_wfc[:], ap["wfc"][:])
    sb_brc = cpool.tile([H, 1], F32, name="c_brc")
    nc.sync.dma_start(sb_brc[:], ap["brc"][:])
    sb_bgc = cpool.tile([H, 1], F32, name="c_bgc")
    nc.sync.dma_start(sb_bgc[:], ap["bgc"][:])
    sb_blc = cpool.tile([H, 1], F32, name="c_blc")
    nc.sync.dma_start(sb_blc[:], ap["blc"][:])
    sb_bfc = cpool.tile([NCLS, 1], F32, name="c_bfc")
    nc.sync.dma_start(sb_bfc[:], ap["bfc"][:])

    sb_ident = cpool.tile([128, 128], F32, name="c_ident")
    nc.sync.dma_start(sb_ident[:], ap["ident"][:])

    l_out = cpool.tile([128, BPC * 3 * NCLS], F32, name="c_lout")  # (128, 576)

    AF = mybir.ActivationFunctionType
    OP = mybir.AluOpType

    # ---- per-dialogue pipeline ----
    for b in range(BPC):
        sb_xt = []
        for ch in range(2):
            t = io.tile([100, SEQ], MMDT, name=f"xt{ch}")
            nc.sync.dma_start(t[:], ap["xt"][b, ch * 100:(ch + 1) * 100, :])
            sb_xt.append(t)
        sb_xn = []
        for k, (u0, uk) in enumerate(UT):
            t = io.tile([128, D], F32, name=f"xn{k}")
            nc.sync.dma_start(t[:uk, :], ap["xn"][b, u0:u0 + uk, :])
            sb_xn.append(t)
        sb_mc = {}
        for s in range(2):
            for k, (u0, uk) in enumerate(UT):
                t = io.tile([128, 1], F32, name=f"mc{s}{k}")
                nc.sync.dma_start(t[:uk, :], ap["msk"][s, b, u0:u0 + uk].unsqueeze(1))
                sb_mc[(s, k)] = t
        # target-speaker mask broadcast to 128 partitions (select mask);
        # SWDGE queue so the strided broadcast doesn't block HWDGE input loads
        sb_tmb = io.tile([H, SEQ], F32, name="tmb", bufs=3)
        nc.gpsimd.dma_start(
            sb_tmb[:], ap["msk"][0, b, :].unsqueeze(0).partition_broadcast(H))

        # scale + exp
        sb_p = []
        for k, (u0, uk) in enumerate(UT):
            pscale = ps_big.tile([128, SEQ], F32, name="pbig", tag="pbig")
            for ch in range(2):
                _mmr(nc, pscale[:uk, :], sb_watt[ch][:, u0:u0 + uk], sb_xt[ch][:],
                     start=(ch == 0), stop=(ch == 1))
            p = wk.tile([128, SEQ], F32, name=f"p{k}")
            nc.scalar.activation(p[:uk, :], pscale[:uk, :], AF.Exp)
            sb_p.append(p)

        # Shat_{dir} (direction-masked exp scores) with row-sum accumulators;
        # source-speaker mask folds into xr instead (saves half the stt ops)
        sb_s = {}
        acc = []
        for k, (u0, uk) in enumerate(UT):
            a2 = wk.tile([128, 2], F32, name=f"acc{k}")
            acc.append(a2)
            for dd in range(2):
                st = spool.tile([128, SEQ], MMDT, name="shat", tag="shat")
                nc.vector.scalar_tensor_tensor(
                    st[:uk, :], sb_p[k][:uk, :], 1.0,
                    sb_dir[(f"dir{dd}", k)][:uk, :],
                    op0=OP.mult, op1=OP.mult,
                    accum_out=a2[:uk, dd:dd + 1],
                )
                sb_s[(dd, k)] = st

        # sums -> recip -> xr_sig = x * (mask_sig / sums)
        sb_xr = {}
        for k, (u0, uk) in enumerate(UT):
            sm = wk.tile([128, 1], F32, name=f"sm{k}")
            nc.vector.reduce_sum(sm[:uk, :], acc[k][:uk, :], axis=mybir.AxisListType.X)
            rc = wk.tile([128, 1], F32, name=f"rc{k}")
            nc.vector.reciprocal(rc[:uk, :], sm[:uk, :])
            for s in range(2):
                rm = wk.tile([128, 1], F32, name=f"rm{s}{k}")
                nc.vector.tensor_tensor(rm[:uk, :], rc[:uk, :], sb_mc[(s, k)][:uk, :],
                                        op=OP.mult)
                xr = wk.tile([128, D], MMDT, name=f"xr{s}{k}")
                nc.vector.tensor_scalar_mul(xr[:uk, :], sb_xn[k][:uk, :], rm[:uk, :])
                sb_xr[(s, k)] = xr

        # G_{sig,dir} banded aggregation (contract over u)
        sb_g = {}
        gi = 0
        for s in range(2):
            for dd in range(2):
                for ch in range(2):
                    pg0 = ps_big.tile([128, SEQ], F32, name="pbig", tag="pbig")
                    pg = pg0[:100, :]
                    for k, (u0, uk) in enumerate(UT):
                        _mmr(nc, pg[:, :], sb_xr[(s, k)][:uk, ch * 100:(ch + 1) * 100],
                             sb_s[(dd, k)][:uk, :], start=(k == 0), stop=(k == 2))
                    g = gpool.tile([100, SEQ], MMDT, name="gsb", tag="gsb")
                    if gi % 2 == 0:
                        nc.scalar.copy(g[:], pg[:])
                    else:
                        nc.vector.tensor_copy(g[:], pg[:])
                    sb_g[(s, dd, ch)] = g
                    gi += 1

        # projections into both target-speaker candidates
        ph1 = []
        for tau in range(2):
            pt = ps_big.tile([H, SEQ], F32, name="pbig", tag="pbig")
            first = True
            for s in range(2):
                for dd in range(2):
                    r = s * 4 + tau * 2 + dd
                    for ch in range(2):
                        _mmr(nc, pt[:, :], sb_w8[ch][:, r * H:(r + 1) * H],
                             sb_g[(s, dd, ch)][:], start=first,
                             stop=(s == 1 and dd == 1 and ch == 1))
                        first = False
            ph1.append(pt)

        proot = ps_big.tile([H, SEQ], F32, name="pbig", tag="pbig")
        for ch in range(2):
            _mmr(nc, proot[:, :], sb_root[:, ch * H:(ch + 1) * H], sb_xt[ch][:],
                 start=(ch == 0), stop=(ch == 1))

        sb_h1 = wk.tile([H, SEQ], F32, name="h1")
        nc.scalar.copy(sb_h1[:], ph1[1][:])
        nc.vector.copy_predicated(sb_h1[:], sb_tmb.bitcast(mybir.dt.int32)[:],
                                  ph1[0][:])
        sb_h1f = wk.tile([H, SEQ], MMDT, name="h1f")
        nc.vector.scalar_tensor_tensor(
            sb_h1f[:], sb_h1[:], sb_brc[:], proot[:], op0=OP.add, op1=OP.add)

        # qT = h1^T W2 (direct transposed layout), then h2
        sb_qt = []
        for k, (u0, uk) in enumerate(UT):
            pq = ps_qt.tile([128, H], F32, name="pqt", tag="pqt")
            nc.tensor.matmul(pq[:uk, :], sb_h1f[:, u0:u0 + uk], sb_w2[:],
                             start=True, stop=True)
            qt = wk.tile([128, H], MMDT, name=f"qt{k}")
            if k == 0:
                nc.scalar.copy(qt[:uk, :], pq[:uk, :])
            else:
                nc.vector.tensor_copy(qt[:uk, :], pq[:uk, :])
            sb_qt.append(qt)

        ph2 = ps_big.tile([H, SEQ], F32, name="pbig", tag="pbig")
        _mmr(nc, ph2[:, :], sb_w1[:], sb_h1f[:], start=True, stop=False)
        for k, (u0, uk) in enumerate(UT):
            _mmr(nc, ph2[:, :], sb_qt[k][:uk, :], sb_dir[("winm", k)][:uk, :],
                 start=False, stop=(k == 2))
        sb_h2 = wk.tile([H, SEQ], MMDT, name="h2")
        nc.scalar.activation(sb_h2[:], ph2[:], AF.Identity, bias=sb_bgc[:])

        phid = ps_big.tile([H, SEQ], F32, name="pbig", tag="pbig")
        for ch in range(2):
            _mmr(nc, phid[:, :], sb_wlind[:, ch * H:(ch + 1) * H], sb_xt[ch][:],
                 start=(ch == 0), stop=False)
        _mmr(nc, phid[:, :], sb_wlinh[:], sb_h2[:], start=False, stop=True)
        sb_hid = wk.tile([H, SEQ], MMDT, name="hid")
        nc.scalar.activation(sb_hid[:], phid[:], AF.Relu, bias=sb_blc[:])

        plg0 = ps_big.tile([128, SEQ], F32, name="plg", tag="plg", bufs=1)
        plg = plg0[:NCLS, :]
        _mmr(nc, plg[:, :], sb_wfc[:], sb_hid[:], start=True, stop=True)
        sb_lg = wk.tile([NCLS, SEQ], F32, name="lg")
        nc.scalar.activation(sb_lg[:], plg[:], AF.Identity, bias=sb_bfc[:])

        plt = ps_lt.tile([128, 3 * NCLS], F32, name="plt", tag="plt")
        for k, (u0, uk) in enumerate(UT):
            nc.tensor.transpose(plt[:uk, k * NCLS:(k + 1) * NCLS],
                                sb_lg[:, u0:u0 + uk], sb_ident[:NCLS, :NCLS])
        nc.vector.tensor_copy(l_out[:, b * 18:b * 18 + 12], plt[:, 0:12])
        nc.vector.tensor_copy(l_out[:44, b * 18 + 12:b * 18 + 18], plt[:44, 12:18])

    # ---- stage 2: batched log-softmax over classes + output DMA ----
    G = BPC * 3  # 96 groups of 6 classes
    l3 = l_out.rearrange("p (g c) -> p g c", c=NCLS)
    m96 = cpool.tile([128, G], F32, name="c_m96")
    nc.vector.reduce_max(m96[:], l3, axis=mybir.AxisListType.X)
    esb = cpool.tile([128, G * NCLS], F32, name="c_esb")
    e3 = esb.rearrange("p (g c) -> p g c", c=NCLS)
    for c in range(NCLS):
        nc.vector.tensor_tensor(e3[:, :, c], l3[:, :, c], m96[:], op=OP.subtract)
    e2sb = cpool.tile([128, G * NCLS], F32, name="c_e2sb")
    nc.scalar.activation(e2sb[:], esb[:], AF.Exp)
    s96 = cpool.tile([128, G], F32, name="c_s96")
    nc.vector.reduce_sum(s96[:], e2sb.rearrange("p (g c) -> p g c", c=NCLS),
                         axis=mybir.AxisListType.X)
    lnz = cpool.tile([128, G], F32, name="c_lnz")
    nc.scalar.activation(lnz[:], s96[:], AF.Ln)
    lsm = cpool.tile([128, G], F32, name="c_lsm")
    nc.vector.tensor_tensor(lsm[:], m96[:], lnz[:], op=OP.add)
    osb = cpool.tile([128, G * NCLS], F32, name="c_osb")
    o3 = osb.rearrange("p (g c) -> p g c", c=NCLS)
    for c in range(NCLS):
        nc.vector.tensor_tensor(o3[:, :, c], l3[:, :, c], lsm[:], op=OP.subtract)

    ov = out.rearrange("(b t) c -> b t c", b=BPC)
    o4 = osb.rearrange("p (b k c) -> p b k c", b=BPC, k=3)
    for k, (u0, uk) in enumerate(UT):
        nc.sync.dma_start(
            ov[:, u0:u0 + uk, :].transpose([1, 0, 2]),
            o4[:uk, :, k, :],
        )



def _host_prep(inputs):
    feats = np.asarray(inputs["features"], dtype=np.float32)    # (300,256,200)
    spk = np.asarray(inputs["speakers"])                        # (300,256)
    W_att = np.asarray(inputs["W_att"], dtype=np.float32)
    basis = np.asarray(inputs["basis"], dtype=np.float32)
    comp = np.asarray(inputs["comp"], dtype=np.float32)
    root = np.asarray(inputs["root"], dtype=np.float32)
    bias_r = np.asarray(inputs["bias_r"], dtype=np.float32)
    W1 = np.asarray(inputs["W1"], dtype=np.float32)
    W2 = np.asarray(inputs["W2"], dtype=np.float32)
    b_gc = np.asarray(inputs["b_gc"], dtype=np.float32)
    W_lin = np.asarray(inputs["W_lin"], dtype=np.float32)
    b_lin = np.asarray(inputs["b_lin"], dtype=np.float32)
    W_fc = np.asarray(inputs["W_fc"], dtype=np.float32)
    b_fc = np.asarray(inputs["b_fc"], dtype=np.float32)

    i = np.arange(SEQ)[:, None]
    j = np.arange(SEQ)[None, :]
    win = (j >= i - WP) & (j <= i + WF)
    dir0 = (win & (i < j)).astype(np.float32)
    dir1 = (win & (i >= j)).astype(np.float32)
    winm = win.astype(np.float32)

    w8 = np.einsum("rb,bdh->rdh", comp, basis).astype(np.float32).reshape(8, 2, 100, H)
    watt = W_att.reshape(2, 100, SEQ)
    rootm = root.reshape(2, 100, H)
    wlind = np.ascontiguousarray(W_lin[:D]).reshape(2, 100, H)
    wlinh = np.ascontiguousarray(W_lin[D:])

    shared = {
        "dir0": dir0, "dir1": dir1, "winm": winm,
        "watt": np.ascontiguousarray(watt),
        "w8": np.ascontiguousarray(w8),
        "rootm": np.ascontiguousarray(rootm),
        "w1m": W1, "w2m": W2,
        "wlind": wlind, "wlinh": wlinh, "wfc": W_fc,
        "ident": np.eye(128, dtype=np.float32),
        "brc": bias_r.reshape(H, 1), "bgc": b_gc.reshape(H, 1),
        "blc": b_lin.reshape(H, 1), "bfc": b_fc.reshape(NCLS, 1),
    }

    in_maps = []
    for c in range(NCORES):
        bs = slice(c * BPC, (c + 1) * BPC)
        fb = feats[:, bs, :]                                    # (300,32,200)
        xt = np.ascontiguousarray(fb.transpose(1, 2, 0))        # (32,200,300)
        xn = np.ascontiguousarray(fb.transpose(1, 0, 2))        # (32,300,200)
        sp = spk[:, bs].T                                       # (32,300)
        msk = np.stack([(sp == 0), (sp == 1)]).astype(np.float32)
        m = {"xt": xt, "xn": xn, "msk": msk}
        m.update(shared)
        in_maps.append(m)
    return in_maps


def get_program():
    if "nc" not in _CACHE:
        _CACHE["nc"] = _build_program()
    return _CACHE["nc"]


def kernel(**inputs):
    nc = get_program()
    in_maps = _host_prep(inputs)
    res = bass_utils.run_bass_kernel_spmd(nc, in_maps, core_ids=list(range(NCORES)))
    return np.concatenate([res.results[c]["out"] for c in range(NCORES)], axis=0)



# revision 10
# speedup vs baseline: 1.1074x; 1.1074x over previous
"""DialogueGCN forward as a Bass/Tile kernel on 8 TRN2 NeuronCores.

Sharding: data-parallel over dialogues (batch). Each core owns 32 contiguous
dialogues; edges never cross dialogues so all graph aggregation is local.

Key structure (per dialogue; u = source utterance, t = target, band |u-t|<=10):
  P[u,t]   = exp((W_att^T x^T)[u,t])       computed only on the band
  Shat_dd  = P * dir_dd                     (banded, bf16)
  sums[u]  = sum_t P*win  (via stt accums); xr_s = x * (msk_s/sums)   (bf16)
  G_{s,dd}[d,t] = sum_u xr_s[u,d] Shat_dd[u,t]   banded matmuls (bf16)
  h1_tau   = sum_{s,dd,ch} w8^T G  (+root^T x^T + bias), tau-select by
             target speaker via PE-broadcast mask + copy_predicated
  h2       = W1^T h1 + (h1^T W2)^T-banded-win + b_gc
  hid      = relu(Wlin^T [x;h2] + b);  logits = Wfc^T hid + b
  out      = log_softmax(logits) over 6 classes (batched stage 2)

All inputs are staged in SBUF by ~35 large DMAs (no per-dialogue DMA);
host pre-lays-out all tensors (incl. bf16 casts); output is one raw
(128, 576) DMA that the host reorders.
"""

import numpy as np
import ml_dtypes

import concourse.bass as bass
import concourse.mybir as mybir
import concourse.tile as tile
from concourse import bass_utils

SEQ, BATCH, D, H, NCLS = 300, 256, 200, 128, 6
WP = WF = 10
NCORES = 8
BPC = BATCH // NCORES  # dialogues per core
UT = [(0, 128), (128, 128), (256, 44)]   # u tiles (offset, size)
BND = [(0, 138), (118, 266), (246, 300)]  # per-u-tile t band [L, R)
BW = 148  # padded band width for constant mask tiles
F32 = mybir.dt.float32
F32R = mybir.dt.float32r
BF16 = mybir.dt.bfloat16
NPBF16 = ml_dtypes.bfloat16

# column-split plan for banded accumulation into a 300-col psum:
# (k, c0, c1, start, stop) with c0/c1 global t coords
GSPLIT = [
    (0, 0, 118, True, True),
    (0, 118, 138, True, False),
    (1, 118, 138, False, True),
    (1, 138, 246, True, True),
    (1, 246, 266, True, False),
    (2, 246, 266, False, True),
    (2, 266, 300, True, True),
]

_CACHE = {}


def _split_multiwaits(nc, max_waits=1):
    """walrus in this container rejects >1 sem wait on an instruction
    ("Too many sync wait commands"); hoist extras onto preceding NOPs."""
    n = 0
    for f in nc.m.functions:
        for b in f.blocks:
            newlist = []
            changed = False
            for ins in b.instructions:
                si = ins.sync_info
                if si is not None and si.on_wait is not None and len(si.on_wait) > max_waits:
                    waits = list(si.on_wait)
                    for w in waits[max_waits:]:
                        n += 1
                        nop = mybir.InstNoOp(name=f"waitsplit-{n}", ins=[], outs=[])
                        nop.engine = ins.engine
                        nop.sync_info = mybir.SyncInfo(on_wait=[w], on_update=[])
                        newlist.append(nop)
                        nc.inst_map[nop.name] = nop
                    ins.sync_info = mybir.SyncInfo(
                        on_wait=waits[:max_waits],
                        on_update=list(si.on_update) if si.on_update else [],
                    )
                    changed = True
                newlist.append(ins)
            if changed:
                b.instructions = newlist
    return n


def _build_program():
    nc = bass.Bass("TRN2", num_devices=NCORES)

    ap = {}
    def din(name, shape, dt=BF16):
        ap[name] = nc.dram_tensor(name, shape, dt, kind="ExternalInput").ap()

    din("xtb", (2, 100, BPC * SEQ))          # x^T d-chunk-major, all dialogues
    din("xnb", (3, 128, BPC * D))            # x u-tile-major (k2 zero-padded)
    din("mskb", (3, 128, 2 * BPC), F32)      # speaker one-hot per u-tile
    din("mskrow", (1, BPC * SEQ), F32)       # speaker-0 mask, dialogue-major row
    din("dirb", (2, 3, 128, BW), F32)        # banded direction masks
    din("winb", (3, 128, BW))                # banded window mask (bf16)
    din("watt", (2, 100, 384))
    din("w8", (2, 100, 8 * H))
    din("rootm", (2, 100, H))
    din("w1m", (H, H))
    din("w2m", (H, H))
    din("wlind", (2, 100, H))
    din("wlinh", (H, H))
    din("wfc", (H, NCLS))
    din("brc", (H, 1), F32)
    din("bgc", (H, 1), F32)
    din("blc", (H, 1), F32)
    din("bfc", (NCLS, 1), F32)
    din("ident", (NCLS, NCLS), F32)
    out = nc.dram_tensor("out", (128, BPC * 3 * NCLS), F32, kind="ExternalOutput").ap()

    from contextlib import ExitStack
    with tile.TileContext(nc) as tc:
        with ExitStack() as ctx:
            pools = dict(
                cpool=ctx.enter_context(tc.tile_pool(name="const", bufs=1)),
                wk=ctx.enter_context(tc.tile_pool(name="wk", bufs=2)),
                spool=ctx.enter_context(tc.tile_pool(name="spool", bufs=2)),
                gpool=ctx.enter_context(tc.tile_pool(name="gpool", bufs=3)),
                ps_sc=ctx.enter_context(tc.tile_pool(name="ps_sc", bufs=1, space="PSUM")),
                ps_g=ctx.enter_context(tc.tile_pool(name="ps_g", bufs=3, space="PSUM")),
                ps_h1=ctx.enter_context(tc.tile_pool(name="ps_h1", bufs=2, space="PSUM")),
                ps_ms=ctx.enter_context(tc.tile_pool(name="ps_ms", bufs=1, space="PSUM")),
            )
            _body(nc, tc, ap, out, pools)

    _split_multiwaits(nc)
    return nc


def _body(nc, tc, ap, out, pools):
    cpool = pools["cpool"]
    wk = pools["wk"]
    spool = pools["spool"]
    gpool = pools["gpool"]
    ps_sc = pools["ps_sc"]
    ps_g = pools["ps_g"]
    ps_h1 = pools["ps_h1"]
    ps_ms = pools["ps_ms"]

    AF = mybir.ActivationFunctionType
    OP = mybir.AluOpType

    # ---- resident constants / staged inputs ----
    def cload(name, shape, dt, src):
        t = cpool.tile(list(shape), dt, name=f"c_{name}")
        nc.sync.dma_start(t[:], src)
        return t

    sb_xt = [cload(f"xt{ch}", (100, BPC * SEQ), BF16, ap["xtb"][ch])
             for ch in range(2)]
    sb_xn = [cload(f"xn{k}", (128, BPC * D), BF16, ap["xnb"][k])
             for k in range(3)]
    sb_mk = [cload(f"mk{k}", (128, 2 * BPC), F32, ap["mskb"][k])
             for k in range(3)]
    sb_tmb = cpool.tile([128, BPC * SEQ], F32, name="c_tmb")
    nc.gpsimd.dma_start(
        sb_tmb[:], ap["mskrow"][0].unsqueeze(0).partition_broadcast(128))
    sb_dir = {(dd, k): cload(f"dir{dd}_{k}", (128, BW), F32, ap["dirb"][dd, k])
              for dd in range(2) for k in range(3)}
    sb_win = [cload(f"win{k}", (128, BW), BF16, ap["winb"][k]) for k in range(3)]
    sb_watt = [cload(f"watt{ch}", (100, 384), BF16, ap["watt"][ch])
               for ch in range(2)]
    sb_w8 = [cload(f"w8_{ch}", (100, 8 * H), BF16, ap["w8"][ch])
             for ch in range(2)]
    sb_root = [cload(f"root{ch}", (100, H), BF16, ap["rootm"][ch])
               for ch in range(2)]
    sb_w1 = cload("w1", (H, H), BF16, ap["w1m"][:])
    sb_w2 = cload("w2", (H, H), BF16, ap["w2m"][:])
    sb_wlind = [cload(f"wlind{ch}", (100, H), BF16, ap["wlind"][ch])
                for ch in range(2)]
    sb_wlinh = cload("wlinh", (H, H), BF16, ap["wlinh"][:])
    sb_wfc = cload("wfc", (H, NCLS), BF16, ap["wfc"][:])
    sb_brc = cload("brc", (H, 1), F32, ap["brc"][:])
    sb_bgc = cload("bgc", (H, 1), F32, ap["bgc"][:])
    sb_blc = cload("blc", (H, 1), F32, ap["blc"][:])
    sb_bfc = cload("bfc", (NCLS, 1), F32, ap["bfc"][:])
    sb_id = cload("ident", (NCLS, NCLS), F32, ap["ident"][:])

    l_out = cpool.tile([128, BPC * 3 * NCLS], F32, name="c_lout")  # (128, 576)

    # ---- per-dialogue pipeline ----
    for b in range(BPC):
        # -- stage A: scale -> P -> Shat/sums -> xr --
        psc = ps_sc.tile([128, 340], F32, name="psc", tag="psc")
        for k, (u0, uk) in enumerate(UT):
            L, R = BND[k]
            cof = [0, 138, 286][k]
            for ch in range(2):
                nc.tensor.matmul(
                    psc[:, cof:cof + (R - L)],
                    sb_watt[ch][:, k * 128:(k + 1) * 128],
                    sb_xt[ch][:, b * SEQ + L:b * SEQ + R],
                    start=(ch == 0), stop=(ch == 1))
        sb_p = wk.tile([128, 340], F32, name="p")
        nc.scalar.activation(sb_p[:], psc[:], AF.Exp)

        # Shat_dd banded (bf16) + per-(dd,k) row-sum accumulators
        sb_s = {}
        acc = wk.tile([128, 6], F32, name="acc")
        for dd in range(2):
            st = spool.tile([128, 340], BF16, name=f"shat{dd}")
            for k, (u0, uk) in enumerate(UT):
                L, R = BND[k]
                cof = [0, 138, 286][k]
                nc.vector.scalar_tensor_tensor(
                    st[:uk, cof:cof + (R - L)], sb_p[:uk, cof:cof + (R - L)],
                    1.0, sb_dir[(dd, k)][:uk, 0:R - L],
                    op0=OP.mult, op1=OP.mult,
                    accum_out=acc[:uk, k * 2 + dd:k * 2 + dd + 1])
            sb_s[dd] = st

        # sums -> 1/sums -> rm_{s} -> xr_s (bf16)
        rc = wk.tile([128, 3], F32, name="rc")
        rm = wk.tile([128, 6], F32, name="rm")
        sb_xr = {}
        for k, (u0, uk) in enumerate(UT):
            sm = wk.tile([128, 1], F32, name=f"sm{k}")
            nc.gpsimd.tensor_tensor(sm[:uk, :], acc[:uk, k * 2:k * 2 + 1],
                                    acc[:uk, k * 2 + 1:k * 2 + 2],
                                    op=OP.add)
            nc.vector.reciprocal(rc[:uk, k:k + 1], sm[:uk, :])
            # rm[:, 2k+s] = msk_s * rc   (strided 2-col AP from mask tile)
            nc.vector.tensor_scalar_mul(
                rm[:uk, k * 2:k * 2 + 2],
                sb_mk[k].rearrange("p (s b) -> p b s", b=BPC)[:uk, b, :],
                rc[:uk, k:k + 1])
            for s in range(2):
                xr = wk.tile([128, D], BF16, name=f"xr{s}{k}")
                nc.vector.tensor_scalar_mul(
                    xr[:uk, :], sb_xn[k][:uk, b * D:(b + 1) * D],
                    rm[:uk, k * 2 + s:k * 2 + s + 1])
                sb_xr[(s, k)] = xr

        # -- stage B: banded G streams + h1 projections --
        sb_g = {}
        gi = 0
        for s in range(2):
            for dd in range(2):
                for ch in range(2):
                    pg = ps_g.tile([128, SEQ], F32, name="pg", tag="pg")
                    for (k, c0, c1, st_, sp_) in GSPLIT:
                        u0, uk = UT[k]
                        L, _ = BND[k]
                        cof = [0, 138, 286][k]
                        nc.tensor.matmul(
                            pg[:100, c0:c1],
                            sb_xr[(s, k)][:uk, ch * 100:(ch + 1) * 100],
                            sb_s[dd][:uk, cof + c0 - L:cof + c1 - L],
                            start=st_, stop=sp_)
                    g = gpool.tile([100, SEQ], BF16, name="g", tag="g")
                    if gi in (0, 2, 4, 6, 7):
                        nc.scalar.copy(g[:], pg[:100, :])
                    else:
                        nc.vector.tensor_copy(g[:], pg[:100, :])
                    sb_g[(s, dd, ch)] = g
                    gi += 1

        ph1 = []
        for tau in range(2):
            pt = ps_h1.tile([H, SEQ], F32, name="ph1", tag="ph1")
            first = True
            for s in range(2):
                for dd in range(2):
                    r = s * 4 + tau * 2 + dd
                    for ch in range(2):
                        nc.tensor.matmul(
                            pt[:, :], sb_w8[ch][:, r * H:(r + 1) * H],
                            sb_g[(s, dd, ch)][:], start=first,
                            stop=(s == 1 and dd == 1 and ch == 1))
                        first = False
            ph1.append(pt)

        proot = ps_ms.tile([H, SEQ], F32, name="proot", tag="msA")
        for ch in range(2):
            nc.tensor.matmul(proot[:, :], sb_root[ch][:],
                             sb_xt[ch][:, b * SEQ:(b + 1) * SEQ],
                             start=(ch == 0), stop=(ch == 1))
        sb_h1 = wk.tile([H, SEQ], F32, name="h1")
        nc.scalar.copy(sb_h1[:], ph1[1][:])
        nc.vector.copy_predicated(
            sb_h1[:],
            sb_tmb.bitcast(mybir.dt.int32)[:, b * SEQ:(b + 1) * SEQ],
            ph1[0][:])
        sb_h1f = wk.tile([H, SEQ], BF16, name="h1f")
        nc.vector.scalar_tensor_tensor(
            sb_h1f[:], sb_h1[:], sb_brc[:], proot[:], op0=OP.add, op1=OP.add)

        # -- stage C: qT then h2 --
        pqt = ps_ms.tile([128, 3 * H], F32, name="pqt", tag="msA")
        for k, (u0, uk) in enumerate(UT):
            nc.tensor.matmul(pqt[:uk, k * H:(k + 1) * H],
                             sb_h1f[:, u0:u0 + uk], sb_w2[:],
                             start=True, stop=True)
        sb_qt = wk.tile([128, 3 * H], BF16, name="qt")
        nc.scalar.copy(sb_qt[:, 0:2 * H], pqt[:, 0:2 * H])
        nc.scalar.copy(sb_qt[:44, 2 * H:3 * H], pqt[:44, 2 * H:3 * H])

        ph2 = ps_h1.tile([H, SEQ], F32, name="ph1", tag="ph1")
        nc.tensor.matmul(ph2[:, :], sb_w1[:], sb_h1f[:], start=True, stop=False)
        for (k, c0, c1, st_, sp_) in GSPLIT:
            u0, uk = UT[k]
            L, _ = BND[k]
            nc.tensor.matmul(ph2[:, c0:c1], sb_qt[:uk, k * H:(k + 1) * H],
                             sb_win[k][:uk, c0 - L:c1 - L],
                             start=False, stop=sp_)
        sb_h2 = wk.tile([H, SEQ], BF16, name="h2")
        nc.scalar.activation(sb_h2[:], ph2[:], AF.Identity, bias=sb_bgc[:])

        # -- stage D: hidden, logits, transpose into l_out --
        phid = ps_g.tile([128, SEQ], F32, name="pg", tag="pg")
        for ch in range(2):
            nc.tensor.matmul(phid[:H, :], sb_wlind[ch][:],
                             sb_xt[ch][:, b * SEQ:(b + 1) * SEQ],
                             start=(ch == 0), stop=False)
        nc.tensor.matmul(phid[:H, :], sb_wlinh[:], sb_h2[:],
                         start=False, stop=True)
        sb_hid = wk.tile([H, SEQ], BF16, name="hid")
        nc.scalar.activation(sb_hid[:], phid[:H, :], AF.Relu, bias=sb_blc[:])

        plg = ps_g.tile([128, SEQ], F32, name="pg", tag="pg")
        nc.tensor.matmul(plg[:NCLS, :], sb_wfc[:], sb_hid[:],
                         start=True, stop=True)
        sb_lg = wk.tile([NCLS, SEQ], F32, name="lg")
        nc.scalar.activation(sb_lg[:], plg[:NCLS, :], AF.Identity, bias=sb_bfc[:])

        plt = ps_ms.tile([128, 3 * NCLS], F32, name="plt", tag="msB")
        for k, (u0, uk) in enumerate(UT):
            nc.tensor.transpose(plt[:uk, k * NCLS:(k + 1) * NCLS],
                                sb_lg[:, u0:u0 + uk], sb_id[:])
        nc.vector.tensor_copy(l_out[:, b * 18:b * 18 + 12], plt[:, 0:12])
        nc.vector.tensor_copy(l_out[:44, b * 18 + 12:b * 18 + 18],
                              plt[:44, 12:18])

    # ---- stage 2: batched log-softmax over classes + single output DMA ----
    G = BPC * 3  # 96 groups of 6 classes
    l3 = l_out.rearrange("p (g c) -> p g c", c=NCLS)
    m96 = cpool.tile([128, G], F32, name="c_m96")
    nc.vector.reduce_max(m96[:], l3, axis=mybir.AxisListType.X)
    esb = cpool.tile([128, G * NCLS], F32, name="c_esb")
    e3 = esb.rearrange("p (g c) -> p g c", c=NCLS)
    for c in range(NCLS):
        nc.vector.tensor_tensor(e3[:, :, c], l3[:, :, c], m96[:], op=OP.subtract)
    e2sb = cpool.tile([128, G * NCLS], F32, name="c_e2sb")
    nc.scalar.activation(e2sb[:], esb[:], AF.Exp)
    s96 = cpool.tile([128, G], F32, name="c_s96")
    nc.vector.reduce_sum(s96[:], e2sb.rearrange("p (g c) -> p g c", c=NCLS),
                         axis=mybir.AxisListType.X)
    lnz = cpool.tile([128, G], F32, name="c_lnz")
    nc.scalar.activation(lnz[:], s96[:], AF.Ln)
    lsm = cpool.tile([128, G], F32, name="c_lsm")
    nc.vector.tensor_tensor(lsm[:], m96[:], lnz[:], op=OP.add)
    osb = cpool.tile([128, G * NCLS], F32, name="c_osb")
    o3 = osb.rearrange("p (g c) -> p g c", c=NCLS)
    for c in range(NCLS):
        nc.vector.tensor_tensor(o3[:, :, c], l3[:, :, c], lsm[:], op=OP.subtract)
    nc.sync.dma_start(out[:, :], osb[:])


def _host_prep(inputs):
    feats = np.asarray(inputs["features"], dtype=np.float32)    # (300,256,200)
    spk = np.asarray(inputs["speakers"])                        # (300,256)
    W_att = np.asarray(inputs["W_att"], dtype=np.float32)
    basis = np.asarray(inputs["basis"], dtype=np.float32)
    comp = np.asarray(inputs["comp"], dtype=np.float32)
    root = np.asarray(inputs["root"], dtype=np.float32)
    bias_r = np.asarray(inputs["bias_r"], dtype=np.float32)
    W1 = np.asarray(inputs["W1"], dtype=np.float32)
    W2 = np.asarray(inputs["W2"], dtype=np.float32)
    b_gc = np.asarray(inputs["b_gc"], dtype=np.float32)
    W_lin = np.asarray(inputs["W_lin"], dtype=np.float32)
    b_lin = np.asarray(inputs["b_lin"], dtype=np.float32)
    W_fc = np.asarray(inputs["W_fc"], dtype=np.float32)
    b_fc = np.asarray(inputs["b_fc"], dtype=np.float32)

    def bf(a):
        return np.ascontiguousarray(a).astype(NPBF16)

    i = np.arange(SEQ)[:, None]
    j = np.arange(SEQ)[None, :]
    win = (j >= i - WP) & (j <= i + WF)
    dir0 = (win & (i < j)).astype(np.float32)
    dir1 = (win & (i >= j)).astype(np.float32)
    winm = win.astype(np.float32)

    dirb = np.zeros((2, 3, 128, BW), np.float32)
    winb = np.zeros((3, 128, BW), np.float32)
    for k, (u0, uk) in enumerate(UT):
        L, R = BND[k]
        dirb[0, k, :uk, :R - L] = dir0[u0:u0 + uk, L:R]
        dirb[1, k, :uk, :R - L] = dir1[u0:u0 + uk, L:R]
        winb[k, :uk, :R - L] = winm[u0:u0 + uk, L:R]

    w = np.einsum("rb,bdh->rdh", comp, basis).astype(np.float32)  # (8,200,128)
    w8 = w.transpose(1, 0, 2).reshape(2, 100, 8 * H)

    shared = {
        "dirb": dirb, "winb": bf(winb),
        "watt": bf(np.concatenate(
            [W_att.reshape(2, 100, SEQ),
             np.zeros((2, 100, 384 - SEQ), np.float32)], axis=2)),
        "w8": bf(w8),
        "rootm": bf(root.reshape(2, 100, H)),
        "w1m": bf(W1), "w2m": bf(W2),
        "wlind": bf(W_lin[:D].reshape(2, 100, H)),
        "wlinh": bf(W_lin[D:]), "wfc": bf(W_fc),
        "brc": bias_r.reshape(H, 1), "bgc": b_gc.reshape(H, 1),
        "blc": b_lin.reshape(H, 1), "bfc": b_fc.reshape(NCLS, 1),
        "ident": np.eye(NCLS, dtype=np.float32),
    }

    in_maps = []
    for c in range(NCORES):
        bs = slice(c * BPC, (c + 1) * BPC)
        fb = feats[:, bs, :]                                    # (300,32,200)
        sp = spk[:, bs]                                         # (300,32)
        xtb = bf(fb.transpose(2, 1, 0).reshape(2, 100, BPC * SEQ))
        xnb = np.zeros((3, 128, BPC * D), NPBF16)
        mskb = np.zeros((3, 128, 2 * BPC), np.float32)
        for k, (u0, uk) in enumerate(UT):
            xnb[k, :uk] = bf(fb[u0:u0 + uk].reshape(uk, BPC * D))
            mm = np.stack([(sp[u0:u0 + uk] == 0), (sp[u0:u0 + uk] == 1)], 1)
            mskb[k, :uk] = mm.astype(np.float32).reshape(uk, 2 * BPC)
        mskrow = (sp.T == 0).astype(np.float32).reshape(1, BPC * SEQ)
        m = {"xtb": xtb, "xnb": xnb, "mskb": mskb, "mskrow": mskrow}
        m.update(shared)
        in_maps.append(m)
    return in_maps


def get_program():
    if "nc" not in _CACHE:
        _CACHE["nc"] = _build_program()
    return _CACHE["nc"]


def kernel(**inputs):
    nc = get_program()
    in_maps = _host_prep(inputs)
    res = bass_utils.run_bass_kernel_spmd(nc, in_maps, core_ids=list(range(NCORES)))
    full = np.empty((NCORES * BPC * SEQ, NCLS), np.float32)
    for c in range(NCORES):
        osb = res.results[c]["out"]                     # (128, 576)
        o4 = osb.reshape(128, BPC, 3, NCLS)
        base = c * BPC * SEQ
        for k, (u0, uk) in enumerate(UT):
            for b in range(BPC):
                full[base + b * SEQ + u0:base + b * SEQ + u0 + uk, :] = \
                    o4[:uk, b, k, :]
    return full


# revision 11
# speedup vs baseline: 1.1187x; 1.0102x over previous
"""DialogueGCN forward as a Bass/Tile kernel on 8 TRN2 NeuronCores.

Sharding: data-parallel over dialogues (batch). Each core owns 32 contiguous
dialogues; edges never cross dialogues so all graph aggregation is local.

Key structure (per dialogue; u = source utterance, t = target, band |u-t|<=10):
  P[u,t]   = exp((W_att^T x^T)[u,t])       computed only on the band
  Shat_dd  = P * dir_dd                     (banded, bf16)
  sums[u]  = sum_t P*win  (via stt accums); xr_s = x * (msk_s/sums)   (bf16)
  G_{s,dd}[d,t] = sum_u xr_s[u,d] Shat_dd[u,t]   banded matmuls (bf16)
  h1_tau   = sum_{s,dd,ch} w8^T G  (+root^T x^T + bias), tau-select by
             target speaker via PE-broadcast mask + copy_predicated
  h2       = W1^T h1 + (h1^T W2)^T-banded-win + b_gc
  hid      = relu(Wlin^T [x;h2] + b);  logits = Wfc^T hid + b
  out      = log_softmax(logits) over 6 classes (batched stage 2)

All inputs are staged in SBUF by ~35 large DMAs (no per-dialogue DMA);
host pre-lays-out all tensors (incl. bf16 casts); output is one raw
(128, 576) DMA that the host reorders.
"""

import numpy as np
import ml_dtypes

import concourse.bass as bass
import concourse.mybir as mybir
import concourse.tile as tile
from concourse import bass_utils

SEQ, BATCH, D, H, NCLS = 300, 256, 200, 128, 6
WP = WF = 10
NCORES = 8
BPC = BATCH // NCORES  # dialogues per core
UT = [(0, 128), (128, 128), (256, 44)]   # u tiles (offset, size)
BND = [(0, 138), (118, 266), (246, 300)]  # per-u-tile t band [L, R)
BW = 148  # padded band width for constant mask tiles
F32 = mybir.dt.float32
F32R = mybir.dt.float32r
BF16 = mybir.dt.bfloat16
NPBF16 = ml_dtypes.bfloat16

# column-split plan for banded accumulation into a 300-col psum:
# (k, c0, c1, start, stop) with c0/c1 global t coords
GSPLIT = [
    (0, 0, 118, True, True),
    (0, 118, 138, True, False),
    (1, 118, 138, False, True),
    (1, 138, 246, True, True),
    (1, 246, 266, True, False),
    (2, 246, 266, False, True),
    (2, 266, 300, True, True),
]

_CACHE = {}


def _split_multiwaits(nc, max_waits=1):
    """walrus in this container rejects >1 sem wait on an instruction
    ("Too many sync wait commands"); hoist extras onto preceding NOPs."""
    n = 0
    for f in nc.m.functions:
        for b in f.blocks:
            newlist = []
            changed = False
            for ins in b.instructions:
                si = ins.sync_info
                if si is not None and si.on_wait is not None and len(si.on_wait) > max_waits:
                    waits = list(si.on_wait)
                    for w in waits[max_waits:]:
                        n += 1
                        nop = mybir.InstNoOp(name=f"waitsplit-{n}", ins=[], outs=[])
                        nop.engine = ins.engine
                        nop.sync_info = mybir.SyncInfo(on_wait=[w], on_update=[])
                        newlist.append(nop)
                        nc.inst_map[nop.name] = nop
                    ins.sync_info = mybir.SyncInfo(
                        on_wait=waits[:max_waits],
                        on_update=list(si.on_update) if si.on_update else [],
                    )
                    changed = True
                newlist.append(ins)
            if changed:
                b.instructions = newlist
    return n


def _build_program():
    nc = bass.Bass("TRN2", num_devices=NCORES)

    ap = {}
    def din(name, shape, dt=BF16):
        ap[name] = nc.dram_tensor(name, shape, dt, kind="ExternalInput").ap()

    din("xtb", (2, 100, BPC * SEQ))          # x^T d-chunk-major, all dialogues
    din("xnb", (3, 128, BPC * D))            # x u-tile-major (k2 zero-padded)
    din("mskb", (3, 128, 2 * BPC), F32)      # speaker one-hot per u-tile
    din("mskrow", (1, BPC * SEQ), F32)       # speaker-0 mask, dialogue-major row
    din("dirb", (2, 3, 128, BW), F32)        # banded direction masks
    din("winb", (3, 128, BW))                # banded window mask (bf16)
    din("watt", (2, 100, 384))
    din("w8", (2, 100, 8 * H))
    din("rootm", (2, 100, H))
    din("w1m", (H, H))
    din("w2m", (H, H))
    din("wlind", (2, 100, H))
    din("wlinh", (H, H))
    din("wfc", (H, NCLS))
    din("brc", (H, 1), F32)
    din("bgc", (H, 1), F32)
    din("blc", (H, 1), F32)
    din("bfc", (NCLS, 1), F32)
    din("ident", (NCLS, NCLS), F32)
    out = nc.dram_tensor("out", (128, BPC * 3 * NCLS), F32, kind="ExternalOutput").ap()

    from contextlib import ExitStack
    with tile.TileContext(nc) as tc:
        with ExitStack() as ctx:
            pools = dict(
                cpool=ctx.enter_context(tc.tile_pool(name="const", bufs=1)),
                wk=ctx.enter_context(tc.tile_pool(name="wk", bufs=2)),
                xpool=ctx.enter_context(tc.tile_pool(name="xpool", bufs=3)),
                fpool=ctx.enter_context(tc.tile_pool(name="fpool", bufs=3)),
                spool=ctx.enter_context(tc.tile_pool(name="spool", bufs=3)),
                gpool=ctx.enter_context(tc.tile_pool(name="gpool", bufs=3)),
                ps_sc=ctx.enter_context(tc.tile_pool(name="ps_sc", bufs=1, space="PSUM")),
                ps_g=ctx.enter_context(tc.tile_pool(name="ps_g", bufs=3, space="PSUM")),
                ps_h1=ctx.enter_context(tc.tile_pool(name="ps_h1", bufs=2, space="PSUM")),
                ps_ms=ctx.enter_context(tc.tile_pool(name="ps_ms", bufs=1, space="PSUM")),
            )
            _body(nc, tc, ap, out, pools)

    _split_multiwaits(nc)
    return nc


def _body(nc, tc, ap, out, pools):
    cpool = pools["cpool"]
    wk = pools["wk"]
    spool = pools["spool"]
    gpool = pools["gpool"]
    ps_sc = pools["ps_sc"]
    ps_g = pools["ps_g"]
    ps_h1 = pools["ps_h1"]
    ps_ms = pools["ps_ms"]

    AF = mybir.ActivationFunctionType
    OP = mybir.AluOpType

    # ---- resident constants / staged inputs ----
    def cload(name, shape, dt, src):
        t = cpool.tile(list(shape), dt, name=f"c_{name}")
        nc.sync.dma_start(t[:], src)
        return t

    sb_xt = [cload(f"xt{ch}", (100, BPC * SEQ), BF16, ap["xtb"][ch])
             for ch in range(2)]
    sb_xn = [cload(f"xn{k}", (128, BPC * D), BF16, ap["xnb"][k])
             for k in range(3)]
    sb_mk = [cload(f"mk{k}", (128, 2 * BPC), F32, ap["mskb"][k])
             for k in range(3)]
    sb_tmb = cpool.tile([128, BPC * SEQ], F32, name="c_tmb")
    nc.gpsimd.dma_start(
        sb_tmb[:], ap["mskrow"][0].unsqueeze(0).partition_broadcast(128))
    sb_dir = {(dd, k): cload(f"dir{dd}_{k}", (128, BW), F32, ap["dirb"][dd, k])
              for dd in range(2) for k in range(3)}
    sb_win = [cload(f"win{k}", (128, BW), BF16, ap["winb"][k]) for k in range(3)]
    sb_watt = [cload(f"watt{ch}", (100, 384), BF16, ap["watt"][ch])
               for ch in range(2)]
    sb_w8 = [cload(f"w8_{ch}", (100, 8 * H), BF16, ap["w8"][ch])
             for ch in range(2)]
    sb_root = [cload(f"root{ch}", (100, H), BF16, ap["rootm"][ch])
               for ch in range(2)]
    sb_w1 = cload("w1", (H, H), BF16, ap["w1m"][:])
    sb_w2 = cload("w2", (H, H), BF16, ap["w2m"][:])
    sb_wlind = [cload(f"wlind{ch}", (100, H), BF16, ap["wlind"][ch])
                for ch in range(2)]
    sb_wlinh = cload("wlinh", (H, H), BF16, ap["wlinh"][:])
    sb_wfc = cload("wfc", (H, NCLS), BF16, ap["wfc"][:])
    sb_brc = cload("brc", (H, 1), F32, ap["brc"][:])
    sb_bgc = cload("bgc", (H, 1), F32, ap["bgc"][:])
    sb_blc = cload("blc", (H, 1), F32, ap["blc"][:])
    sb_bfc = cload("bfc", (NCLS, 1), F32, ap["bfc"][:])
    sb_id = cload("ident", (NCLS, NCLS), F32, ap["ident"][:])

    l_out = cpool.tile([128, BPC * 3 * NCLS], F32, name="c_lout")  # (128, 576)

    # ---- per-dialogue pipeline, software-pipelined 3 stages deep ----
    xpool = pools["xpool"]
    fpool = pools["fpool"]

    def S1(b):
        """scale -> P -> Shat/sums -> xr. Returns cross-stage tiles."""
        psc = ps_sc.tile([128, 340], F32, name="psc", tag="psc")
        for k, (u0, uk) in enumerate(UT):
            L, R = BND[k]
            cof = [0, 138, 286][k]
            for ch in range(2):
                nc.tensor.matmul(
                    psc[:, cof:cof + (R - L)],
                    sb_watt[ch][:, k * 128:(k + 1) * 128],
                    sb_xt[ch][:, b * SEQ + L:b * SEQ + R],
                    start=(ch == 0), stop=(ch == 1))
        sb_p = wk.tile([128, 340], F32, name="p")
        nc.scalar.activation(sb_p[:], psc[:], AF.Exp)

        sb_s = {}
        acc = wk.tile([128, 6], F32, name="acc")
        for dd in range(2):
            st = spool.tile([128, 340], BF16, name=f"shat{dd}")
            for k, (u0, uk) in enumerate(UT):
                L, R = BND[k]
                cof = [0, 138, 286][k]
                nc.vector.scalar_tensor_tensor(
                    st[:uk, cof:cof + (R - L)], sb_p[:uk, cof:cof + (R - L)],
                    1.0, sb_dir[(dd, k)][:uk, 0:R - L],
                    op0=OP.mult, op1=OP.mult,
                    accum_out=acc[:uk, k * 2 + dd:k * 2 + dd + 1])
            sb_s[dd] = st

        rc = wk.tile([128, 3], F32, name="rc")
        rm = wk.tile([128, 6], F32, name="rm")
        sb_xr = {}
        for k, (u0, uk) in enumerate(UT):
            sm = wk.tile([128, 1], F32, name=f"sm{k}")
            nc.gpsimd.tensor_tensor(sm[:uk, :], acc[:uk, k * 2:k * 2 + 1],
                                    acc[:uk, k * 2 + 1:k * 2 + 2],
                                    op=OP.add)
            nc.vector.reciprocal(rc[:uk, k:k + 1], sm[:uk, :])
            nc.vector.tensor_scalar_mul(
                rm[:uk, k * 2:k * 2 + 2],
                sb_mk[k].rearrange("p (s b) -> p b s", b=BPC)[:uk, b, :],
                rc[:uk, k:k + 1])
            for s in range(2):
                xr = xpool.tile([128, D], BF16, name=f"xr{s}{k}")
                nc.vector.tensor_scalar_mul(
                    xr[:uk, :], sb_xn[k][:uk, b * D:(b + 1) * D],
                    rm[:uk, k * 2 + s:k * 2 + s + 1])
                sb_xr[(s, k)] = xr
        return sb_s, sb_xr

    def S2(b, sb_s, sb_xr):
        """banded G streams + h1 projections + tau-select. Returns h1f."""
        sb_g = {}
        gi = 0
        for s in range(2):
            for dd in range(2):
                for ch in range(2):
                    pg = ps_g.tile([128, SEQ], F32, name="pg", tag="pg")
                    for (k, c0, c1, st_, sp_) in GSPLIT:
                        u0, uk = UT[k]
                        L, _ = BND[k]
                        cof = [0, 138, 286][k]
                        nc.tensor.matmul(
                            pg[:100, c0:c1],
                            sb_xr[(s, k)][:uk, ch * 100:(ch + 1) * 100],
                            sb_s[dd][:uk, cof + c0 - L:cof + c1 - L],
                            start=st_, stop=sp_)
                    g = gpool.tile([100, SEQ], BF16, name="g", tag="g")
                    if gi in (0, 2, 4, 6, 7):
                        nc.scalar.copy(g[:], pg[:100, :])
                    else:
                        nc.vector.tensor_copy(g[:], pg[:100, :])
                    sb_g[(s, dd, ch)] = g
                    gi += 1

        ph1 = []
        for tau in range(2):
            pt = ps_h1.tile([H, SEQ], F32, name="ph1", tag="ph1")
            first = True
            for s in range(2):
                for dd in range(2):
                    r = s * 4 + tau * 2 + dd
                    for ch in range(2):
                        nc.tensor.matmul(
                            pt[:, :], sb_w8[ch][:, r * H:(r + 1) * H],
                            sb_g[(s, dd, ch)][:], start=first,
                            stop=(s == 1 and dd == 1 and ch == 1))
                        first = False
            ph1.append(pt)

        proot = ps_ms.tile([H, SEQ], F32, name="proot", tag="msA")
        for ch in range(2):
            nc.tensor.matmul(proot[:, :], sb_root[ch][:],
                             sb_xt[ch][:, b * SEQ:(b + 1) * SEQ],
                             start=(ch == 0), stop=(ch == 1))
        sb_h1 = wk.tile([H, SEQ], F32, name="h1")
        nc.scalar.copy(sb_h1[:], ph1[1][:])
        nc.vector.copy_predicated(
            sb_h1[:],
            sb_tmb.bitcast(mybir.dt.int32)[:, b * SEQ:(b + 1) * SEQ],
            ph1[0][:])
        sb_h1f = fpool.tile([H, SEQ], BF16, name="h1f")
        nc.vector.scalar_tensor_tensor(
            sb_h1f[:], sb_h1[:], sb_brc[:], proot[:], op0=OP.add, op1=OP.add)
        return sb_h1f

    def S3(b, sb_h1f):
        """qT/h2/hidden/logits/transpose into l_out."""
        pqt = ps_ms.tile([128, 3 * H], F32, name="pqt", tag="msA")
        for k, (u0, uk) in enumerate(UT):
            nc.tensor.matmul(pqt[:uk, k * H:(k + 1) * H],
                             sb_h1f[:, u0:u0 + uk], sb_w2[:],
                             start=True, stop=True)
        sb_qt = wk.tile([128, 3 * H], BF16, name="qt")
        nc.scalar.copy(sb_qt[:, 0:2 * H], pqt[:, 0:2 * H])
        nc.scalar.copy(sb_qt[:44, 2 * H:3 * H], pqt[:44, 2 * H:3 * H])

        ph2 = ps_h1.tile([H, SEQ], F32, name="ph1", tag="ph1")
        nc.tensor.matmul(ph2[:, :], sb_w1[:], sb_h1f[:], start=True, stop=False)
        for (k, c0, c1, st_, sp_) in GSPLIT:
            u0, uk = UT[k]
            L, _ = BND[k]
            nc.tensor.matmul(ph2[:, c0:c1], sb_qt[:uk, k * H:(k + 1) * H],
                             sb_win[k][:uk, c0 - L:c1 - L],
                             start=False, stop=sp_)
        sb_h2 = wk.tile([H, SEQ], BF16, name="h2")
        nc.scalar.activation(sb_h2[:], ph2[:], AF.Identity, bias=sb_bgc[:])

        phid = ps_g.tile([128, SEQ], F32, name="pg", tag="pg")
        for ch in range(2):
            nc.tensor.matmul(phid[:H, :], sb_wlind[ch][:],
                             sb_xt[ch][:, b * SEQ:(b + 1) * SEQ],
                             start=(ch == 0), stop=False)
        nc.tensor.matmul(phid[:H, :], sb_wlinh[:], sb_h2[:],
                         start=False, stop=True)
        sb_hid = wk.tile([H, SEQ], BF16, name="hid")
        nc.scalar.activation(sb_hid[:], phid[:H, :], AF.Relu, bias=sb_blc[:])

        plg = ps_g.tile([128, SEQ], F32, name="pg", tag="pg")
        nc.tensor.matmul(plg[:NCLS, :], sb_wfc[:], sb_hid[:],
                         start=True, stop=True)
        sb_lg = wk.tile([NCLS, SEQ], F32, name="lg")
        nc.scalar.activation(sb_lg[:], plg[:NCLS, :], AF.Identity, bias=sb_bfc[:])

        plt = ps_ms.tile([128, 3 * NCLS], F32, name="plt", tag="msB")
        for k, (u0, uk) in enumerate(UT):
            nc.tensor.transpose(plt[:uk, k * NCLS:(k + 1) * NCLS],
                                sb_lg[:, u0:u0 + uk], sb_id[:])
        nc.vector.tensor_copy(l_out[:, b * 18:b * 18 + 12], plt[:, 0:12])
        nc.vector.tensor_copy(l_out[:44, b * 18 + 12:b * 18 + 18],
                              plt[:44, 12:18])

    state = {}
    for i in range(BPC + 2):
        if i < BPC:
            state[i] = S1(i)
        if 1 <= i <= BPC:
            state[i - 1] = S2(i - 1, *state[i - 1])
        if i >= 2:
            S3(i - 2, state[i - 2])
            del state[i - 2]

    # ---- stage 2: batched log-softmax over classes + single output DMA ----
    G = BPC * 3  # 96 groups of 6 classes
    l3 = l_out.rearrange("p (g c) -> p g c", c=NCLS)
    m96 = cpool.tile([128, G], F32, name="c_m96")
    nc.vector.reduce_max(m96[:], l3, axis=mybir.AxisListType.X)
    esb = cpool.tile([128, G * NCLS], F32, name="c_esb")
    e3 = esb.rearrange("p (g c) -> p g c", c=NCLS)
    for c in range(NCLS):
        nc.vector.tensor_tensor(e3[:, :, c], l3[:, :, c], m96[:], op=OP.subtract)
    e2sb = cpool.tile([128, G * NCLS], F32, name="c_e2sb")
    nc.scalar.activation(e2sb[:], esb[:], AF.Exp)
    s96 = cpool.tile([128, G], F32, name="c_s96")
    nc.vector.reduce_sum(s96[:], e2sb.rearrange("p (g c) -> p g c", c=NCLS),
                         axis=mybir.AxisListType.X)
    lnz = cpool.tile([128, G], F32, name="c_lnz")
    nc.scalar.activation(lnz[:], s96[:], AF.Ln)
    lsm = cpool.tile([128, G], F32, name="c_lsm")
    nc.vector.tensor_tensor(lsm[:], m96[:], lnz[:], op=OP.add)
    osb = cpool.tile([128, G * NCLS], F32, name="c_osb")
    o3 = osb.rearrange("p (g c) -> p g c", c=NCLS)
    for c in range(NCLS):
        nc.vector.tensor_tensor(o3[:, :, c], l3[:, :, c], lsm[:], op=OP.subtract)
    nc.sync.dma_start(out[:, :], osb[:])


def _host_prep(inputs):
    feats = np.asarray(inputs["features"], dtype=np.float32)    # (300,256,200)
    spk = np.asarray(inputs["speakers"])                        # (300,256)
    W_att = np.asarray(inputs["W_att"], dtype=np.float32)
    basis = np.asarray(inputs["basis"], dtype=np.float32)
    comp = np.asarray(inputs["comp"], dtype=np.float32)
    root = np.asarray(inputs["root"], dtype=np.float32)
    bias_r = np.asarray(inputs["bias_r"], dtype=np.float32)
    W1 = np.asarray(inputs["W1"], dtype=np.float32)
    W2 = np.asarray(inputs["W2"], dtype=np.float32)
    b_gc = np.asarray(inputs["b_gc"], dtype=np.float32)
    W_lin = np.asarray(inputs["W_lin"], dtype=np.float32)
    b_lin = np.asarray(inputs["b_lin"], dtype=np.float32)
    W_fc = np.asarray(inputs["W_fc"], dtype=np.float32)
    b_fc = np.asarray(inputs["b_fc"], dtype=np.float32)

    def bf(a):
        return np.ascontiguousarray(a).astype(NPBF16)

    i = np.arange(SEQ)[:, None]
    j = np.arange(SEQ)[None, :]
    win = (j >= i - WP) & (j <= i + WF)
    dir0 = (win & (i < j)).astype(np.float32)
    dir1 = (win & (i >= j)).astype(np.float32)
    winm = win.astype(np.float32)

    dirb = np.zeros((2, 3, 128, BW), np.float32)
    winb = np.zeros((3, 128, BW), np.float32)
    for k, (u0, uk) in enumerate(UT):
        L, R = BND[k]
        dirb[0, k, :uk, :R - L] = dir0[u0:u0 + uk, L:R]
        dirb[1, k, :uk, :R - L] = dir1[u0:u0 + uk, L:R]
        winb[k, :uk, :R - L] = winm[u0:u0 + uk, L:R]

    w = np.einsum("rb,bdh->rdh", comp, basis).astype(np.float32)  # (8,200,128)
    w8 = w.transpose(1, 0, 2).reshape(2, 100, 8 * H)

    shared = {
        "dirb": dirb, "winb": bf(winb),
        "watt": bf(np.concatenate(
            [W_att.reshape(2, 100, SEQ),
             np.zeros((2, 100, 384 - SEQ), np.float32)], axis=2)),
        "w8": bf(w8),
        "rootm": bf(root.reshape(2, 100, H)),
        "w1m": bf(W1), "w2m": bf(W2),
        "wlind": bf(W_lin[:D].reshape(2, 100, H)),
        "wlinh": bf(W_lin[D:]), "wfc": bf(W_fc),
        "brc": bias_r.reshape(H, 1), "bgc": b_gc.reshape(H, 1),
        "blc": b_lin.reshape(H, 1), "bfc": b_fc.reshape(NCLS, 1),
        "ident": np.eye(NCLS, dtype=np.float32),
    }

    in_maps = []
    for c in range(NCORES):
        bs = slice(c * BPC, (c + 1) * BPC)
        fb = feats[:, bs, :]                                    # (300,32,200)
        sp = spk[:, bs]                                         # (300,32)
        xtb = bf(fb.transpose(2, 1, 0).reshape(2, 100, BPC * SEQ))
        xnb = np.zeros((3, 128, BPC * D), NPBF16)
        mskb = np.zeros((3, 128, 2 * BPC), np.float32)
        for k, (u0, uk) in enumerate(UT):
            xnb[k, :uk] = bf(fb[u0:u0 + uk].reshape(uk, BPC * D))
            mm = np.stack([(sp[u0:u0 + uk] == 0), (sp[u0:u0 + uk] == 1)], 1)
            mskb[k, :uk] = mm.astype(np.float32).reshape(uk, 2 * BPC)
        mskrow = (sp.T == 0).astype(np.float32).reshape(1, BPC * SEQ)
        m = {"xtb": xtb, "xnb": xnb, "mskb": mskb, "mskrow": mskrow}
        m.update(shared)
        in_maps.append(m)
    return in_maps


def get_program():
    if "nc" not in _CACHE:
        _CACHE["nc"] = _build_program()
    return _CACHE["nc"]


def kernel(**inputs):
    nc = get_program()
    in_maps = _host_prep(inputs)
    res = bass_utils.run_bass_kernel_spmd(nc, in_maps, core_ids=list(range(NCORES)))
    full = np.empty((NCORES * BPC * SEQ, NCLS), np.float32)
    for c in range(NCORES):
        osb = res.results[c]["out"]                     # (128, 576)
        o4 = osb.reshape(128, BPC, 3, NCLS)
        base = c * BPC * SEQ
        for k, (u0, uk) in enumerate(UT):
            for b in range(BPC):
                full[base + b * SEQ + u0:base + b * SEQ + u0 + uk, :] = \
                    o4[:uk, b, k, :]
    return full


# revision 12
# speedup vs baseline: 1.8089x; 1.6169x over previous
"""DialogueGCN forward as a Bass/Tile kernel on 8 TRN2 NeuronCores.

Sharding: data-parallel over dialogues (batch). Each core owns 32 contiguous
dialogues; edges never cross dialogues so all graph aggregation is local.

Key structure (per dialogue; u = source utterance, t = target, band |u-t|<=10):
  P[u,t]   = exp((W_att^T x^T)[u,t])       computed only on the band
  Shat_dd  = P * dir_dd                     (banded, bf16)
  sums[u]  = sum_t P*win  (via stt accums); xr_s = x * (msk_s/sums)   (bf16)
  G_{s,dd}[d,t] = sum_u xr_s[u,d] Shat_dd[u,t]   banded matmuls (bf16)
  h1_tau   = sum_{s,dd,ch} w8^T G  (+root^T x^T + bias), tau-select by
             target speaker via PE-broadcast mask + copy_predicated
  h2       = W1^T h1 + (h1^T W2)^T-banded-win + b_gc
  hid      = relu(Wlin^T [x;h2] + b);  logits = Wfc^T hid + b
  out      = log_softmax(logits) over 6 classes (batched stage 2)

All inputs are staged in SBUF by ~35 large DMAs (no per-dialogue DMA);
host pre-lays-out all tensors (incl. bf16 casts); output is one raw
(128, 576) DMA that the host reorders.
"""

import numpy as np
import ml_dtypes

import concourse.bass as bass
import concourse.mybir as mybir
import concourse.tile as tile
from concourse import bass_utils

SEQ, BATCH, D, H, NCLS = 300, 256, 200, 128, 6
WP = WF = 10
NCORES = 8
BPC = BATCH // NCORES  # dialogues per core
UT = [(0, 128), (128, 128), (256, 44)]   # u tiles (offset, size)
BND = [(0, 138), (118, 266), (246, 300)]  # per-u-tile t band [L, R)
BW = 148  # padded band width for constant mask tiles
F32 = mybir.dt.float32
F32R = mybir.dt.float32r
BF16 = mybir.dt.bfloat16
NPBF16 = ml_dtypes.bfloat16

# column-split plan for banded accumulation into a 300-col psum:
# (k, c0, c1, start, stop) with c0/c1 global t coords
GSPLIT = [
    (0, 0, 118, True, True),
    (0, 118, 138, True, False),
    (1, 118, 138, False, True),
    (1, 138, 246, True, True),
    (1, 246, 266, True, False),
    (2, 246, 266, False, True),
    (2, 266, 300, True, True),
]

_CACHE = {}


def _split_multiwaits(nc, max_waits=1):
    """walrus in this container rejects >1 sem wait on an instruction
    ("Too many sync wait commands"); hoist extras onto preceding NOPs."""
    n = 0
    for f in nc.m.functions:
        for b in f.blocks:
            newlist = []
            changed = False
            for ins in b.instructions:
                si = ins.sync_info
                if si is not None and si.on_wait is not None and len(si.on_wait) > max_waits:
                    waits = list(si.on_wait)
                    for w in waits[max_waits:]:
                        n += 1
                        nop = mybir.InstNoOp(name=f"waitsplit-{n}", ins=[], outs=[])
                        nop.engine = ins.engine
                        nop.sync_info = mybir.SyncInfo(on_wait=[w], on_update=[])
                        newlist.append(nop)
                        nc.inst_map[nop.name] = nop
                    ins.sync_info = mybir.SyncInfo(
                        on_wait=waits[:max_waits],
                        on_update=list(si.on_update) if si.on_update else [],
                    )
                    changed = True
                newlist.append(ins)
            if changed:
                b.instructions = newlist
    return n


def _build_program():
    nc = bass.Bass("TRN2", num_devices=NCORES)

    ap = {}
    def din(name, shape, dt=BF16):
        ap[name] = nc.dram_tensor(name, shape, dt, kind="ExternalInput").ap()

    din("xtb", (2, 101, BPC * SEQ))          # x^T d-chunk-major, all dialogues
    din("xnb", (3, 128, BPC * D))            # x u-tile-major (k2 zero-padded)
    din("mskb", (3, 128, 2 * BPC), F32)      # speaker one-hot per u-tile
    din("mskrow", (1, BPC * SEQ), F32)       # speaker-0 mask, dialogue-major row
    din("dirb", (2, 3, 128, BW), F32)        # banded direction masks
    din("winb", (3, 128, BW))                # banded window mask (bf16)
    din("watt", (2, 100, 384))
    din("w8", (2, 100, 8 * H))
    din("rootm", (2, 101, H))
    din("w1m", (H, H))
    din("w2m", (H, H))
    din("wlind", (2, 101, H))
    din("wlinh", (H, H))
    din("wfc", (H, NCLS))
    din("bgc", (H, 1), F32)
    din("blc", (H, 1), F32)
    din("bfc", (NCLS, 1), F32)
    din("ident", (NCLS, NCLS), F32)
    out = nc.dram_tensor("out", (128, BPC * 3 * NCLS), F32, kind="ExternalOutput").ap()

    from contextlib import ExitStack
    with tile.TileContext(nc) as tc:
        with ExitStack() as ctx:
            pools = dict(
                cpool=ctx.enter_context(tc.tile_pool(name="const", bufs=1)),
                wk=ctx.enter_context(tc.tile_pool(name="wk", bufs=2)),
                xpool=ctx.enter_context(tc.tile_pool(name="xpool", bufs=3)),
                fpool=ctx.enter_context(tc.tile_pool(name="fpool", bufs=3)),
                spool=ctx.enter_context(tc.tile_pool(name="spool", bufs=3)),
                gpool=ctx.enter_context(tc.tile_pool(name="gpool", bufs=3)),
                ps_sc=ctx.enter_context(tc.tile_pool(name="ps_sc", bufs=1, space="PSUM")),
                ps_g=ctx.enter_context(tc.tile_pool(name="ps_g", bufs=3, space="PSUM")),
                ps_h1=ctx.enter_context(tc.tile_pool(name="ps_h1", bufs=2, space="PSUM")),
                ps_ms=ctx.enter_context(tc.tile_pool(name="ps_ms", bufs=1, space="PSUM")),
            )
            _body(nc, tc, ap, out, pools)

    _split_multiwaits(nc)
    return nc


def _body(nc, tc, ap, out, pools):
    cpool = pools["cpool"]
    wk = pools["wk"]
    spool = pools["spool"]
    gpool = pools["gpool"]
    ps_sc = pools["ps_sc"]
    ps_g = pools["ps_g"]
    ps_h1 = pools["ps_h1"]
    ps_ms = pools["ps_ms"]

    AF = mybir.ActivationFunctionType
    OP = mybir.AluOpType

    # ---- resident constants / staged inputs ----
    def cload(name, shape, dt, src):
        t = cpool.tile(list(shape), dt, name=f"c_{name}")
        nc.sync.dma_start(t[:], src)
        return t

    sb_xt = [cload(f"xt{ch}", (101, BPC * SEQ), BF16, ap["xtb"][ch])
             for ch in range(2)]
    sb_xn = [cload(f"xn{k}", (128, BPC * D), BF16, ap["xnb"][k])
             for k in range(3)]
    sb_mk = [cload(f"mk{k}", (128, 2 * BPC), F32, ap["mskb"][k])
             for k in range(3)]
    sb_tmb = cpool.tile([128, BPC * SEQ], F32, name="c_tmb")
    nc.gpsimd.dma_start(
        sb_tmb[:], ap["mskrow"][0].unsqueeze(0).partition_broadcast(128))
    sb_dir = {(dd, k): cload(f"dir{dd}_{k}", (128, BW), F32, ap["dirb"][dd, k])
              for dd in range(2) for k in range(3)}
    sb_win = [cload(f"win{k}", (128, BW), BF16, ap["winb"][k]) for k in range(3)]
    sb_watt = [cload(f"watt{ch}", (100, 384), BF16, ap["watt"][ch])
               for ch in range(2)]
    sb_w8 = [cload(f"w8_{ch}", (100, 8 * H), BF16, ap["w8"][ch])
             for ch in range(2)]
    sb_root = [cload(f"root{ch}", (101, H), BF16, ap["rootm"][ch])
               for ch in range(2)]
    sb_w1 = cload("w1", (H, H), BF16, ap["w1m"][:])
    sb_w2 = cload("w2", (H, H), BF16, ap["w2m"][:])
    sb_wlind = [cload(f"wlind{ch}", (101, H), BF16, ap["wlind"][ch])
                for ch in range(2)]
    sb_wlinh = cload("wlinh", (H, H), BF16, ap["wlinh"][:])
    sb_wfc = cload("wfc", (H, NCLS), BF16, ap["wfc"][:])
    sb_bgc = cload("bgc", (H, 1), F32, ap["bgc"][:])
    sb_blc = cload("blc", (H, 1), F32, ap["blc"][:])
    sb_bfc = cload("bfc", (NCLS, 1), F32, ap["bfc"][:])
    sb_id = cload("ident", (NCLS, NCLS), F32, ap["ident"][:])

    l_out = cpool.tile([128, BPC * 3 * NCLS], F32, name="c_lout")  # (128, 576)

    # ---- per-dialogue pipeline, software-pipelined 3 stages deep ----
    xpool = pools["xpool"]
    fpool = pools["fpool"]

    def S1(b):
        """scale -> P -> Shat/sums -> xr. Returns cross-stage tiles."""
        psc = ps_sc.tile([128, 340], F32, name="psc", tag="psc")
        for k, (u0, uk) in enumerate(UT):
            L, R = BND[k]
            cof = [0, 138, 286][k]
            for ch in range(2):
                nc.tensor.matmul(
                    psc[:, cof:cof + (R - L)],
                    sb_watt[ch][:, k * 128:(k + 1) * 128],
                    sb_xt[ch][:100, b * SEQ + L:b * SEQ + R],
                    start=(ch == 0), stop=(ch == 1))
        sb_p = wk.tile([128, 340], F32, name="p")
        nc.scalar.activation(sb_p[:], psc[:], AF.Exp)

        sb_s = {}
        acc = wk.tile([128, 6], F32, name="acc")
        for dd in range(2):
            st = spool.tile([128, 340], BF16, name=f"shat{dd}")
            for k, (u0, uk) in enumerate(UT):
                L, R = BND[k]
                cof = [0, 138, 286][k]
                nc.vector.scalar_tensor_tensor(
                    st[:uk, cof:cof + (R - L)], sb_p[:uk, cof:cof + (R - L)],
                    1.0, sb_dir[(dd, k)][:uk, 0:R - L],
                    op0=OP.mult, op1=OP.mult,
                    accum_out=acc[:uk, k * 2 + dd:k * 2 + dd + 1])
            sb_s[dd] = st

        rc = wk.tile([128, 3], F32, name="rc")
        rm = wk.tile([128, 6], F32, name="rm")
        sb_xr = {}
        for k, (u0, uk) in enumerate(UT):
            sm = wk.tile([128, 1], F32, name=f"sm{k}")
            nc.gpsimd.tensor_tensor(sm[:uk, :], acc[:uk, k * 2:k * 2 + 1],
                                    acc[:uk, k * 2 + 1:k * 2 + 2],
                                    op=OP.add)
            nc.vector.reciprocal(rc[:uk, k:k + 1], sm[:uk, :])
            nc.vector.tensor_scalar_mul(
                rm[:uk, k * 2:k * 2 + 2],
                sb_mk[k].rearrange("p (s b) -> p b s", b=BPC)[:uk, b, :],
                rc[:uk, k:k + 1])
            for s in range(2):
                xr = xpool.tile([128, D], BF16, name=f"xr{s}{k}")
                nc.vector.tensor_scalar_mul(
                    xr[:uk, :], sb_xn[k][:uk, b * D:(b + 1) * D],
                    rm[:uk, k * 2 + s:k * 2 + s + 1])
                sb_xr[(s, k)] = xr
        return sb_s, sb_xr

    def S2(b, sb_s, sb_xr):
        """banded G streams + h1 projections + tau-select. Returns h1f."""
        sb_g = {}
        gi = 0
        for s in range(2):
            for dd in range(2):
                for ch in range(2):
                    pg = ps_g.tile([128, SEQ], F32, name="pg", tag="pg")
                    for (k, c0, c1, st_, sp_) in GSPLIT:
                        u0, uk = UT[k]
                        L, _ = BND[k]
                        cof = [0, 138, 286][k]
                        nc.tensor.matmul(
                            pg[:100, c0:c1],
                            sb_xr[(s, k)][:uk, ch * 100:(ch + 1) * 100],
                            sb_s[dd][:uk, cof + c0 - L:cof + c1 - L],
                            start=st_, stop=sp_)
                    g = gpool.tile([100, SEQ], BF16, name="g", tag="g")
                    if gi in (0, 2, 4, 6):
                        nc.scalar.copy(g[:], pg[:100, :])
                    else:
                        nc.vector.tensor_copy(g[:], pg[:100, :])
                    sb_g[(s, dd, ch)] = g
                    gi += 1

        ph1 = []
        for tau in range(2):
            pt = ps_h1.tile([H, SEQ], F32, name="ph1", tag="ph1")
            first = True
            for s in range(2):
                for dd in range(2):
                    r = s * 4 + tau * 2 + dd
                    for ch in range(2):
                        nc.tensor.matmul(
                            pt[:, :], sb_w8[ch][:, r * H:(r + 1) * H],
                            sb_g[(s, dd, ch)][:], start=first, stop=False)
                        first = False
            for ch in range(2):
                nc.tensor.matmul(pt[:, :], sb_root[ch][:],
                                 sb_xt[ch][:, b * SEQ:(b + 1) * SEQ],
                                 start=False, stop=(ch == 1))
            ph1.append(pt)

        sb_h1f = fpool.tile([H, SEQ], F32R, name="h1f")
        nc.scalar.copy(sb_h1f[:], ph1[1][:])
        nc.vector.copy_predicated(
            sb_h1f[:],
            sb_tmb.bitcast(mybir.dt.int32)[:, b * SEQ:(b + 1) * SEQ],
            ph1[0][:])
        return sb_h1f

    def S3(b, sb_h1f):
        """qT/h2/hidden/logits/transpose into l_out."""
        pqt = ps_ms.tile([128, 3 * H], F32, name="pqt", tag="s3a")
        for k, (u0, uk) in enumerate(UT):
            nc.tensor.matmul(pqt[:uk, k * H:(k + 1) * H],
                             sb_h1f[:, u0:u0 + uk], sb_w2[:],
                             start=True, stop=True)
        sb_qt = wk.tile([128, 3 * H], BF16, name="qt")
        nc.scalar.copy(sb_qt[:, 0:2 * H], pqt[:, 0:2 * H])
        nc.scalar.copy(sb_qt[:44, 2 * H:3 * H], pqt[:44, 2 * H:3 * H])

        ph2 = ps_ms.tile([H, SEQ], F32, name="ph2", tag="s3b")
        nc.tensor.matmul(ph2[:, :], sb_w1[:], sb_h1f[:], start=True, stop=False)
        for (k, c0, c1, st_, sp_) in GSPLIT:
            u0, uk = UT[k]
            L, _ = BND[k]
            nc.tensor.matmul(ph2[:, c0:c1], sb_qt[:uk, k * H:(k + 1) * H],
                             sb_win[k][:uk, c0 - L:c1 - L],
                             start=False, stop=sp_)
        sb_h2 = wk.tile([H, SEQ], BF16, name="h2")
        nc.scalar.activation(sb_h2[:], ph2[:], AF.Identity, bias=sb_bgc[:])

        phid = ps_ms.tile([128, SEQ], F32, name="phid", tag="s3a")
        for ch in range(2):
            nc.tensor.matmul(phid[:H, :], sb_wlind[ch][:],
                             sb_xt[ch][:, b * SEQ:(b + 1) * SEQ],
                             start=(ch == 0), stop=False)
        nc.tensor.matmul(phid[:H, :], sb_wlinh[:], sb_h2[:],
                         start=False, stop=True)
        sb_hid = wk.tile([H, SEQ], BF16, name="hid")
        nc.scalar.activation(sb_hid[:], phid[:H, :], AF.Relu, bias=sb_blc[:])

        plgt = ps_ms.tile([128, 330], F32, name="plgt", tag="s3b")
        nc.tensor.matmul(plgt[:NCLS, 0:SEQ], sb_wfc[:], sb_hid[:],
                         start=True, stop=True)
        sb_lg = wk.tile([NCLS, SEQ], F32, name="lg")
        nc.scalar.activation(sb_lg[:], plgt[:NCLS, 0:SEQ], AF.Identity,
                             bias=sb_bfc[:])

        for k, (u0, uk) in enumerate(UT):
            nc.tensor.transpose(plgt[:uk, 312 + k * NCLS:312 + (k + 1) * NCLS],
                                sb_lg[:, u0:u0 + uk], sb_id[:])
        nc.vector.tensor_copy(l_out[:, b * 18:b * 18 + 12], plgt[:, 312:324])
        nc.vector.tensor_copy(l_out[:44, b * 18 + 12:b * 18 + 18],
                              plgt[:44, 324:330])

    state = {}
    for i in range(BPC + 2):
        if i < BPC:
            state[i] = S1(i)
        if 1 <= i <= BPC:
            state[i - 1] = S2(i - 1, *state[i - 1])
        if i >= 2:
            S3(i - 2, state[i - 2])
            del state[i - 2]

    # ---- stage 2: batched log-softmax over classes + single output DMA ----
    G = BPC * 3  # 96 groups of 6 classes
    l3 = l_out.rearrange("p (g c) -> p g c", c=NCLS)
    m96 = cpool.tile([128, G], F32, name="c_m96")
    nc.vector.reduce_max(m96[:], l3, axis=mybir.AxisListType.X)
    esb = cpool.tile([128, G * NCLS], F32, name="c_esb")
    e3 = esb.rearrange("p (g c) -> p g c", c=NCLS)
    for c in range(NCLS):
        nc.vector.tensor_tensor(e3[:, :, c], l3[:, :, c], m96[:], op=OP.subtract)
    e2sb = cpool.tile([128, G * NCLS], F32, name="c_e2sb")
    nc.scalar.activation(e2sb[:], esb[:], AF.Exp)
    s96 = cpool.tile([128, G], F32, name="c_s96")
    nc.vector.reduce_sum(s96[:], e2sb.rearrange("p (g c) -> p g c", c=NCLS),
                         axis=mybir.AxisListType.X)
    lnz = cpool.tile([128, G], F32, name="c_lnz")
    nc.scalar.activation(lnz[:], s96[:], AF.Ln)
    lsm = cpool.tile([128, G], F32, name="c_lsm")
    nc.vector.tensor_tensor(lsm[:], m96[:], lnz[:], op=OP.add)
    osb = cpool.tile([128, G * NCLS], F32, name="c_osb")
    o3 = osb.rearrange("p (g c) -> p g c", c=NCLS)
    for c in range(NCLS):
        nc.vector.tensor_tensor(o3[:, :, c], l3[:, :, c], lsm[:], op=OP.subtract)
    nc.sync.dma_start(out[:, :], osb[:])


def _host_prep(inputs):
    feats = np.asarray(inputs["features"], dtype=np.float32)    # (300,256,200)
    spk = np.asarray(inputs["speakers"])                        # (300,256)
    W_att = np.asarray(inputs["W_att"], dtype=np.float32)
    basis = np.asarray(inputs["basis"], dtype=np.float32)
    comp = np.asarray(inputs["comp"], dtype=np.float32)
    root = np.asarray(inputs["root"], dtype=np.float32)
    bias_r = np.asarray(inputs["bias_r"], dtype=np.float32)
    W1 = np.asarray(inputs["W1"], dtype=np.float32)
    W2 = np.asarray(inputs["W2"], dtype=np.float32)
    b_gc = np.asarray(inputs["b_gc"], dtype=np.float32)
    W_lin = np.asarray(inputs["W_lin"], dtype=np.float32)
    b_lin = np.asarray(inputs["b_lin"], dtype=np.float32)
    W_fc = np.asarray(inputs["W_fc"], dtype=np.float32)
    b_fc = np.asarray(inputs["b_fc"], dtype=np.float32)

    def bf(a):
        return np.ascontiguousarray(a).astype(NPBF16)

    i = np.arange(SEQ)[:, None]
    j = np.arange(SEQ)[None, :]
    win = (j >= i - WP) & (j <= i + WF)
    dir0 = (win & (i < j)).astype(np.float32)
    dir1 = (win & (i >= j)).astype(np.float32)
    winm = win.astype(np.float32)

    dirb = np.zeros((2, 3, 128, BW), np.float32)
    winb = np.zeros((3, 128, BW), np.float32)
    for k, (u0, uk) in enumerate(UT):
        L, R = BND[k]
        dirb[0, k, :uk, :R - L] = dir0[u0:u0 + uk, L:R]
        dirb[1, k, :uk, :R - L] = dir1[u0:u0 + uk, L:R]
        winb[k, :uk, :R - L] = winm[u0:u0 + uk, L:R]

    w = np.einsum("rb,bdh->rdh", comp, basis).astype(np.float32)  # (8,200,128)
    w8 = w.transpose(1, 0, 2).reshape(2, 100, 8 * H)

    shared = {
        "dirb": dirb, "winb": bf(winb),
        "watt": bf(np.concatenate(
            [W_att.reshape(2, 100, SEQ),
             np.zeros((2, 100, 384 - SEQ), np.float32)], axis=2)),
        "w8": bf(w8),
        "rootm": bf(np.concatenate(
            [root.reshape(2, 100, H),
             np.stack([np.zeros((1, H), np.float32),
                       bias_r.reshape(1, H)])], axis=1)),
        "w1m": bf(W1), "w2m": bf(W2),
        "wlind": bf(np.concatenate(
            [W_lin[:D].reshape(2, 100, H),
             np.stack([np.zeros((1, H), np.float32),
                       b_lin.reshape(1, H)])], axis=1)),
        "wlinh": bf(W_lin[D:]), "wfc": bf(W_fc),
        "bgc": b_gc.reshape(H, 1),
        "blc": b_lin.reshape(H, 1), "bfc": b_fc.reshape(NCLS, 1),
        "ident": np.eye(NCLS, dtype=np.float32),
    }

    in_maps = []
    for c in range(NCORES):
        bs = slice(c * BPC, (c + 1) * BPC)
        fb = feats[:, bs, :]                                    # (300,32,200)
        sp = spk[:, bs]                                         # (300,32)
        xtb = np.zeros((2, 101, BPC * SEQ), NPBF16)
        xtb[:, :100] = bf(fb.transpose(2, 1, 0).reshape(2, 100, BPC * SEQ))
        xtb[1, 100] = NPBF16(1.0)
        xnb = np.zeros((3, 128, BPC * D), NPBF16)
        mskb = np.zeros((3, 128, 2 * BPC), np.float32)
        for k, (u0, uk) in enumerate(UT):
            xnb[k, :uk] = bf(fb[u0:u0 + uk].reshape(uk, BPC * D))
            mm = np.stack([(sp[u0:u0 + uk] == 0), (sp[u0:u0 + uk] == 1)], 1)
            mskb[k, :uk] = mm.astype(np.float32).reshape(uk, 2 * BPC)
        mskrow = (sp.T == 0).astype(np.float32).reshape(1, BPC * SEQ)
        m = {"xtb": xtb, "xnb": xnb, "mskb": mskb, "mskrow": mskrow}
        m.update(shared)
        in_maps.append(m)
    return in_maps


def get_program():
    if "nc" not in _CACHE:
        _CACHE["nc"] = _build_program()
    return _CACHE["nc"]


def kernel(**inputs):
    nc = get_program()
    in_maps = _host_prep(inputs)
    res = bass_utils.run_bass_kernel_spmd(nc, in_maps, core_ids=list(range(NCORES)))
    full = np.empty((NCORES * BPC * SEQ, NCLS), np.float32)
    for c in range(NCORES):
        osb = res.results[c]["out"]                     # (128, 576)
        o4 = osb.reshape(128, BPC, 3, NCLS)
        base = c * BPC * SEQ
        for k, (u0, uk) in enumerate(UT):
            for b in range(BPC):
                full[base + b * SEQ + u0:base + b * SEQ + u0 + uk, :] = \
                    o4[:uk, b, k, :]
    return full
